# revision 27
# baseline (speedup 1.0000x reference)
"""AudioSNN forward pass on 8 Trainium2 NeuronCores (pure data parallel).

Host side: weight re-layout + padding (numpy). Device side: Bass/Tile kernel
per core over a 128-batch shard.
"""
import os
import sys
import numpy as np

for _p in ("/opt/trn_rl_repo", "/root/.axon_site/_ro/trn_rl_repo"):
    if os.path.isdir(_p) and _p not in sys.path:
        sys.path.insert(0, _p)

import ml_dtypes
from contextlib import ExitStack

import concourse.bass as bass
import concourse.tile as tile
from concourse import mybir, bacc
from concourse.bass_utils import run_bass_kernel_spmd

BF16 = mybir.dt.bfloat16
F32 = mybir.dt.float32
GT = mybir.AluOpType.is_gt
ADD = mybir.AluOpType.add
SUB = mybir.AluOpType.subtract
MUL = mybir.AluOpType.mult
SIGMOID = mybir.ActivationFunctionType.Sigmoid

N_CORES = 8
B = 1024
BL = B // N_CORES        # 128 batch per core
H, W = 64, 32            # conv1 image
HP, WP = H + 3, W + 3    # padded (67, 35); conv1 uses a 4x4 tap window
H2, W2 = 32, 16          # conv2 image (after pool1)
H2P, W2P = H2 + 2, W2 + 2  # 34, 18
H3, W3 = 16, 8           # after pool2
NS = 25
BETA = 0.95
SCALE = float(2.0 ** 96)  # sigmoid saturation scale (exact power of two)

BC = 32                  # conv2 batch-chunk
NCHUNK = BL // BC        # 4
P1COLS = H2P * W2P * BC  # 34*18*32 = 19584
GROUPS = BL // 4         # 32 conv1 groups (4 batches each)

# spike engine split: tiles with index % MOD == 0 go to ACT, rest to DVE
ACT_MOD1 = 2
ACT_MOD2 = 2


def _emit_c2pool(nc, spool, c2v, chunk, gt, pw2, g):
    """pool2-h for one 2-pair group: selection matmuls summing the two
    row-partition blocks (in-place into bank 0 of the group tile), then the
    pooled [128 = (h-par, ch), (b, w')] slice is transposed into c2buf."""
    for qq in range(2):
        nc.tensor.matmul(gt[64 * qq:64 * (qq + 1), 0:256], spool[:, 0:64],
                         pw2[:, 256 * qq:256 * (qq + 1)], start=True, stop=True,
                         tile_position=(0, 64 * qq), skip_group_check=True)
    gq = gt[:].rearrange("p (s b w) -> p s w b", s=4, b=BC)
    src = gq[:, 0:1, :, :].squeeze()
    dst = c2v[:, 8 * g:8 * (g + 1), chunk * BC:(chunk + 1) * BC]
    nc.vector.tensor_copy(dst, src)


def build_program():
    nc = bacc.Bacc()

    # ---- inputs (host-preprocessed layouts) ----
    XP = nc.declare_dram_parameter("xp", [BL + 1, HP * WP], BF16, isOutput=False)
    WC1 = nc.declare_dram_parameter("wc1", [128, 128], BF16, isOutput=False)
    B1S = nc.declare_dram_parameter("b1s", [128, 1], F32, isOutput=False)   # -SCALE*thr1
    T1 = nc.declare_dram_parameter("t1", [128, 1], F32, isOutput=False)     # thr1
    WC2R = nc.declare_dram_parameter("wc2r", [96, 4 * 128], BF16, isOutput=False)  # [(dx,ci), (rho, 2row x 64co)]
    SPOOL = nc.declare_dram_parameter("spool", [128, 64], BF16, isOutput=False)   # pool-h selection
    B2S = nc.declare_dram_parameter("b2s", [128, 1], F32, isOutput=False)   # -SCALE*thr2
    T2 = nc.declare_dram_parameter("t2", [128, 1], F32, isOutput=False)     # thr2
    FC1WHI = nc.declare_dram_parameter("fc1whi", [128, 2 * 64 * 128], BF16, isOutput=False)
    FC1WLO = nc.declare_dram_parameter("fc1wlo", [128, 2 * 64 * 128], BF16, isOutput=False)
    FC1B = nc.declare_dram_parameter("fc1b", [128, 2], F32, isOutput=False)
    FC2A = nc.declare_dram_parameter("fc2a", [128, 256], BF16, isOutput=False)  # lhsT hi [u3_low, (chunk, u4)]
    FC2B = nc.declare_dram_parameter("fc2b", [128, 256], BF16, isOutput=False)  # lo
    FC2BIAS = nc.declare_dram_parameter("fc2bias", [128, 1], F32, isOutput=False)
    FC3A = nc.declare_dram_parameter("fc3a", [128, 16], BF16, isOutput=False)   # lhsT hi [u4, 10->16]
    FC3B = nc.declare_dram_parameter("fc3b", [128, 16], BF16, isOutput=False)
    FC3BIAS = nc.declare_dram_parameter("fc3bias", [16, 1], F32, isOutput=False)

    OUT = nc.declare_dram_parameter("out", [16, NS * BL], F32, isOutput=True)

    dbg = os.environ.get("KERNEL_DEBUG", "")
    dbg_outs = {}
    if dbg:
        dbg_outs["p1"] = nc.declare_dram_parameter("dbg_p1", [128, P1COLS], BF16, isOutput=True)
        dbg_outs["c2"] = nc.declare_dram_parameter("dbg_c2", [128, 64 * BL], BF16, isOutput=True)
        dbg_outs["cur3"] = nc.declare_dram_parameter("dbg_cur3", [256, BL], F32, isOutput=True)

    with tile.TileContext(nc) as tc, ExitStack() as top:
        consts = top.enter_context(tc.tile_pool(name="consts", bufs=1))

        wc1 = consts.tile([128, 128], BF16, tag="wc1")
        nc.sync.dma_start(wc1[:], WC1[:])
        b1s = consts.tile([128, 1], F32, tag="b1s")
        nc.sync.dma_start(b1s[:], B1S[:])
        t1 = consts.tile([128, 1], F32, tag="t1")
        nc.sync.dma_start(t1[:], T1[:])
        wc2r = consts.tile([96, 4 * 128], BF16, tag="wc2r")
        nc.sync.dma_start(wc2r[:], WC2R[:])
        spool = consts.tile([128, 64], BF16, tag="spool")
        nc.sync.dma_start(spool[:], SPOOL[:])
        b2s = consts.tile([128, 1], F32, tag="b2s")
        nc.sync.dma_start(b2s[:], B2S[:])
        t2 = consts.tile([128, 1], F32, tag="t2")
        nc.sync.dma_start(t2[:], T2[:])

        # c2 accumulation buffer: [128 = 2x64ch, 64 rounds x 128 batch] bf16
        # (pool2 counts are small integers -> exact in bf16)
        c2buf = top.enter_context(tc.tile_pool(name="c2bufp", bufs=1)).tile(
            [128, 64 * BL], BF16, tag="c2buf")

        # fc1 weights (bf16 hi/lo split); DMAs are issued mid-conv (gpsimd
        # queue) so the 8.4 MB transfer overlaps conv compute without
        # delaying the conv1 input loads.
        fc1w_hi = consts.tile([128, 2 * 64 * 128], BF16, tag="fc1w_hi")
        fc1w_lo = consts.tile([128, 2 * 64 * 128], BF16, tag="fc1w_lo")

        # ------------- conv1 + spike1 + pool1 + conv2 + spike2 + pool2 -------------
        with ExitStack() as convs:
            p1pool = convs.enter_context(tc.tile_pool(name="p1pool", bufs=1))
            xrep_pool = convs.enter_context(tc.tile_pool(name="xrep", bufs=2))
            s1pool = convs.enter_context(tc.tile_pool(name="s1pool", bufs=2))
            pwpool = convs.enter_context(tc.tile_pool(name="pwpool", bufs=2))
            stgpool = convs.enter_context(tc.tile_pool(name="stgpool", bufs=1))
            c1ps = convs.enter_context(tc.tile_pool(name="c1ps", bufs=2, space="PSUM"))
            c2ps = convs.enter_context(tc.tile_pool(name="c2ps", bufs=2, space="PSUM"))
            s2pool = convs.enter_context(tc.tile_pool(name="s2pool", bufs=4))
            pw2pool = convs.enter_context(tc.tile_pool(name="pw2pool", bufs=4))

            # persistent double-buffered P1 + padded pool1 staging; pads are
            # zeroed once and never overwritten afterwards.
            p1bufs = [p1pool.tile([128, P1COLS], BF16, tag=f"p1_{i}", name=f"p1_{i}")
                      for i in range(2)]
            for i in range(2):
                pv = p1bufs[i][:].rearrange("p (b h w) -> p b h w", b=BC, h=H2P, w=W2P)
                nc.vector.memset(pv[0:32, :, 0, :], 0.0)
                nc.vector.memset(pv[0:32, :, H2P - 1, :], 0.0)
            # chunk-wide pool1 staging: [128 = (4b x 32ch), (8 blk x 32h x 18w)]
            stg3 = stgpool.tile([128, 8 * H2 * W2P], BF16, tag="stg3")
            nc.vector.memset(stg3[:], 0.0)

            # explicit xr buffers (rows with dy=3/dx=3 are never re-loaded and
            # must stay finite -> one-time memset)
            XW = H * WP  # 2240
            xrbufs = [xrep_pool.tile([128, XW], BF16, tag=f"xrb_{i}", name=f"xrb_{i}")
                      for i in range(2)]
            for i in range(2):
                nc.vector.memset(xrbufs[i][:], 0.0)

            for chunk in range(NCHUNK):
                # P1: [128 = 4 replicas x 32ch, (b, h2p, w2p)] bf16, pool-sums 0..4
                p1 = p1bufs[chunk % 2]
                p1v = p1[:].rearrange("p (b h w) -> p b h w", b=BC, h=H2P, w=W2P)
                stg = stg3
                stgv = stg[:].rearrange("p (blk q hp v) -> p blk q hp v",
                                        blk=8, q=4, hp=8, v=W2P)

                for gg in range(BC // 8):
                    # xr rows: 64*sub + 4*k + b; each row holds a contiguous
                    # 64x64 window of padded x starting at (dy, dx)
                    xr = xrbufs[(chunk * 4 + gg) % 2]
                    xrv = xr[:].rearrange("p (h w) -> p h w", h=H, w=WP)
                    # rows with dy=3 or dx=3 carry zero weights in wc1 -> left
                    # stale; only 12 of 16 rows per (sub, dy) are loaded.
                    xsrc = XP[:]
                    for sub_ in range(2):
                        for dy in range(3):
                            src = bass.AP(xsrc.tensor,
                                          xsrc.offset + (chunk * BC + gg * 8 + 4 * sub_) * (HP * WP) + dy * WP,
                                          [[1, 3], [HP * WP, 4], [1, XW]])
                            r0 = 64 * sub_ + 16 * dy
                            nc.sync.dma_start(xr[r0:r0 + 12, :], src)

                    for sub in range(2):
                        sb = 64 * sub
                        # conv1 + fused spike1/pool1-w:
                        #   pw = (u_even > thr1) + spike(u_odd)
                        pwsub = pwpool.tile([128, 1024], BF16, tag="pw")  # (4q,16h,16w')
                        pwsubv = pwsub[:].rearrange("p (q h w) -> p q h w", q=4, h=16)
                        for half in range(2):
                            hs = c1ps.tile([128, 1024], F32, tag="c1ps")
                            for qh in range(2):
                                q4 = 2 * half + qh
                                nc.tensor.matmul(hs[:, 512 * qh:512 * (qh + 1)],
                                                 wc1[sb:sb + 64, :],
                                                 xrv[sb:sb + 64, 16 * q4:16 * q4 + 16, 0:W],
                                                 start=True, stop=True,
                                                 tile_position=(sb, 0))
                            hsv = hs[:].rearrange("p (q h w) -> p q h w", q=2, h=16)
                            s1o = s1pool.tile([128, 512], BF16, tag="s1o")
                            s1ov = s1o[:].rearrange("p (q h w) -> p q h w", q=2, h=16)
                            nc.scalar.activation(s1ov, hsv[:, :, :, 1::2], SIGMOID,
                                                 bias=b1s[:], scale=SCALE)
                            nc.vector.scalar_tensor_tensor(
                                pwsubv[:, 2 * half:2 * half + 2], hsv[:, :, :, 0::2],
                                t1[:], s1ov, op0=GT, op1=ADD)
                        # pool1 h-pairs into padded staging block (GpSimd)
                        blk = 2 * gg + sub
                        pwq = pwsub[:].rearrange("p (q hp t w) -> p q hp t w",
                                                 q=4, hp=8, t=2)
                        nc.gpsimd.tensor_tensor(stgv[:, blk, :, :, 1:W2 + 1],
                                                pwq[:, :, :, 0, :], pwq[:, :, :, 1, :],
                                                op=ADD)

                # scatter into P1 replica 0 (interior rows): one DMA per b4
                for b4 in range(4):
                    dst = bass.AP(p1.tensor, p1.offset + b4 * (H2P * W2P) + W2P,
                                  [[P1COLS, 32], [4 * H2P * W2P, 8], [1, H2 * W2P]])
                    nc.sync.dma_start(dst, stg[32 * b4:32 * (b4 + 1), :])
                # replicate P1 block 0 -> blocks 1..2, pre-shifted by dx=j elems
                for rep in range(1, 3):
                    nc.gpsimd.dma_start(p1[32 * rep:32 * (rep + 1), 0:P1COLS - rep],
                                        p1[0:32, rep:P1COLS])
                if dbg and chunk == 0:
                    nc.sync.dma_start(dbg_outs["p1"][:], p1[:])

                # ---- conv2: row-pair scheme, M=128 = (2 rows x 64 ch_out) ----
                # pooled-row-pair q covers output rows (2q, 2q+1); 4 accumulating
                # K=96 matmuls over input rows 2q+rho. Groups of 2 pairs share a
                # 2-bank psum tile; spike2 + pool2-w fuse into one ACT + one STT;
                # pool2-h is a PE matmul against a 2x64 selection matrix, written
                # in-place into the group tile's first bank.
                if chunk == 1:
                    nc.scalar.dma_start(fc1w_hi[:], FC1WHI[:])
                if chunk == 2:
                    nc.scalar.dma_start(fc1w_lo[:], FC1WLO[:])
                c2v = c2buf[:].rearrange("p (r b) -> p r b", b=BL)
                pend = None
                for g in range(8):
                    gt = c2ps.tile([128, 1024], F32, tag="c2g", name=f"c2g_{chunk}_{g}")
                    for qq in range(2):
                        q = 2 * g + qq
                        for rho in range(4):
                            nc.tensor.matmul(gt[:, 512 * qq:512 * (qq + 1)],
                                             wc2r[0:96, 128 * rho:128 * (rho + 1)],
                                             p1v[0:96, :, 2 * q + rho, 0:W2],
                                             start=(rho == 0), stop=(rho == 3))
                    gtv = gt[:].rearrange("p (qq b w) -> p qq b w", qq=2, b=BC)
                    s2o = s2pool.tile([128, 512], BF16, tag="s2o", name=f"s2o_{chunk}_{g}")
                    s2ov = s2o[:].rearrange("p (qq b w) -> p qq b w", qq=2, b=BC)
                    nc.scalar.activation(s2ov, gtv[:, :, :, 1::2], SIGMOID,
                                         bias=b2s[:], scale=SCALE)
                    pw2 = pw2pool.tile([128, 512], BF16, tag="pw2", name=f"pw2_{chunk}_{g}")
                    pw2v = pw2[:].rearrange("p (qq b w) -> p qq b w", qq=2, b=BC)
                    nc.vector.scalar_tensor_tensor(pw2v, gtv[:, :, :, 0::2], t2[:],
                                                   s2ov, op0=GT, op1=ADD)
                    if pend is not None:
                        _emit_c2pool(nc, spool, c2v, chunk, *pend)
                    pend = (gt, pw2, g)
                _emit_c2pool(nc, spool, c2v, chunk, *pend)

        if dbg:
            nc.sync.dma_start(dbg_outs["c2"][:], c2buf[:])

        # ---------------- fc1 (bf16 hi/lo) + LIF ----------------
        with ExitStack() as fcs:
            fc1ps = fcs.enter_context(tc.tile_pool(name="fc1ps", bufs=2, space="PSUM"))
            lifps = fcs.enter_context(tc.tile_pool(name="lifps", bufs=2, space="PSUM"))
            lifc = fcs.enter_context(tc.tile_pool(name="lifc", bufs=1))

            fc1b = consts.tile([128, 2], F32, tag="fc1b")
            nc.sync.dma_start(fc1b[:], FC1B[:])

            cur3c = lifc.tile([128, 2 * BL], F32, tag="cur3c")
            c2r = c2buf[:].rearrange("p (r b) -> p r b", b=BL)
            whiv = fc1w_hi[:].rearrange("p (h r u) -> p h r u", h=2, r=64)
            wlov = fc1w_lo[:].rearrange("p (h r u) -> p h r u", h=2, r=64)
            for h in range(2):
                ps = fc1ps.tile([128, BL], F32, tag="fc1ps", name=f"fc1ps_{h}")
                for r in range(64):
                    nc.tensor.matmul(ps[:], whiv[:, h, r, :], c2r[:, r, :],
                                     start=(r == 0), stop=False)
                    nc.tensor.matmul(ps[:], wlov[:, h, r, :], c2r[:, r, :],
                                     start=False, stop=(r == 63))
                # cur3 = psum + fc1_b (the 1/4 pool scale is folded into weights)
                nc.vector.tensor_scalar(cur3c[:, h * BL:(h + 1) * BL], ps[:],
                                        fc1b[:, h:h + 1], None, op0=ADD)
            if dbg:
                nc.sync.dma_start(dbg_outs["cur3"][0:128, :], cur3c[:, 0:BL])
                nc.sync.dma_start(dbg_outs["cur3"][128:256, :], cur3c[:, BL:2 * BL])

            # LIF weights
            fc2a = consts.tile([128, 256], BF16, tag="fc2a")
            nc.sync.dma_start(fc2a[:], FC2A[:])
            fc2b_w = consts.tile([128, 256], BF16, tag="fc2b_w")
            nc.sync.dma_start(fc2b_w[:], FC2B[:])
            fc2bias = consts.tile([128, 1], F32, tag="fc2bias")
            nc.sync.dma_start(fc2bias[:], FC2BIAS[:])
            fc3a = consts.tile([128, 16], BF16, tag="fc3a")
            nc.sync.dma_start(fc3a[:], FC3A[:])
            fc3b_w = consts.tile([128, 16], BF16, tag="fc3b_w")
            nc.sync.dma_start(fc3b_w[:], FC3B[:])
            fc3bias = consts.tile([16, 1], F32, tag="fc3bias")
            nc.sync.dma_start(fc3bias[:], FC3BIAS[:])

            # LIF chains, software-pipelined per step across engines:
            #   layer3 mem update: DVE; spk3: ACT (sigmoid saturation trick)
            #   fc2: PE; cur4 psum->SBUF copy: ACT
            #   layer4 chain: GpSimd (fc2 bias folded into the mem update)
            #   fc3: PE; layer5 chain: DVE (reads p5 psum directly)
            mem3 = lifc.tile([128, 2 * BL], F32, tag="mem3")
            t3 = lifc.tile([128, 2 * BL], F32, tag="t3")
            spk3buf = lifc.tile([128, NS * 2 * BL], BF16, tag="spk3buf")
            mem4 = lifc.tile([128, BL], F32, tag="mem4")
            t4 = lifc.tile([128, BL], F32, tag="t4")
            cur4buf = lifc.tile([128, NS * BL], F32, tag="cur4buf")
            spk4buf = lifc.tile([128, NS * BL], BF16, tag="spk4buf")
            mem5 = lifc.tile([16, BL], F32, tag="mem5")
            t5 = lifc.tile([16, BL], F32, tag="t5")
            outstage = lifc.tile([16, NS * BL], F32, tag="outstage")
            zero3 = lifc.tile([128, 2 * BL], BF16, tag="zero3")
            zero4 = lifc.tile([128, BL], BF16, tag="zero4")
            zero5 = lifc.tile([16, BL], F32, tag="zero5")
            for t_ in (mem3, mem4, mem5, zero3, zero4, zero5):
                nc.vector.memset(t_[:], 0.0)
            # bias tile for sigmoid(SCALE*(x-1)) spike trick
            nsig3 = lifc.tile([128, 1], F32, tag="nsig3")
            nc.vector.memset(nsig3[:], -float(SCALE))

            cur5buf = lifc.tile([16, NS * BL], F32, tag="cur5buf")
            for st in range(NS):
                # ---- layer 3: STT (DVE), sub (GpSimd), spike (ACT) ----
                nc.vector.scalar_tensor_tensor(t3[:], mem3[:], BETA, cur3c[:],
                                               op0=MUL, op1=ADD)
                prev3 = zero3[:] if st == 0 else spk3buf[:, (st - 1) * 2 * BL:st * 2 * BL]
                nc.gpsimd.tensor_tensor(mem3[:], t3[:], prev3, op=SUB)
                s3 = spk3buf[:, st * 2 * BL:(st + 1) * 2 * BL]
                nc.scalar.activation(s3, mem3[:], SIGMOID, bias=nsig3[:], scale=SCALE)
                # ---- fc2 (PE) ----
                p4 = lifps.tile([128, BL], F32, tag="p4", name=f"p4_{st}")
                s3a = spk3buf[:, st * 2 * BL:st * 2 * BL + BL]
                s3b = spk3buf[:, st * 2 * BL + BL:(st + 1) * 2 * BL]
                nc.tensor.matmul(p4[:], fc2a[:, 0:128], s3a, start=True, stop=False)
                nc.tensor.matmul(p4[:], fc2a[:, 128:256], s3b, start=False, stop=False)
                nc.tensor.matmul(p4[:], fc2b_w[:, 0:128], s3a, start=False, stop=False)
                nc.tensor.matmul(p4[:], fc2b_w[:, 128:256], s3b, start=False, stop=True)
                # ---- layer 4: bias (DVE), STT (DVE), sub (GpSimd), spike (ACT) ----
                cur4 = cur4buf[:, st * BL:(st + 1) * BL]
                nc.vector.tensor_scalar(cur4, p4[:], fc2bias[:, 0:1], None, op0=ADD)
                nc.vector.scalar_tensor_tensor(t4[:], mem4[:], BETA, cur4,
                                               op0=MUL, op1=ADD)
                prev4 = zero4[:] if st == 0 else spk4buf[:, (st - 1) * BL:st * BL]
                nc.gpsimd.tensor_tensor(mem4[:], t4[:], prev4, op=SUB)
                s4 = spk4buf[:, st * BL:(st + 1) * BL]
                nc.scalar.activation(s4, mem4[:], SIGMOID, bias=nsig3[:], scale=SCALE)
                # ---- fc3 (PE) + layer 5 (DVE/GpSimd/ACT) ----
                p5 = lifps.tile([16, BL], F32, tag="p5", name=f"p5_{st}")
                nc.tensor.matmul(p5[:], fc3a[:], s4, start=True, stop=False)
                nc.tensor.matmul(p5[:], fc3b_w[:], s4, start=False, stop=True)
                cur5 = cur5buf[:, st * BL:(st + 1) * BL]
                nc.vector.tensor_scalar(cur5, p5[:], fc3bias[:, 0:1], None, op0=ADD)
                nc.vector.scalar_tensor_tensor(t5[:], mem5[:], BETA, cur5,
                                               op0=MUL, op1=ADD)
                prev5 = zero5[:] if st == 0 else outstage[:, (st - 1) * BL:st * BL]
                nc.gpsimd.tensor_tensor(mem5[:], t5[:], prev5, op=SUB)
                nc.scalar.activation(outstage[:, st * BL:(st + 1) * BL],
                                     mem5[:], SIGMOID, bias=nsig3[0:16, :], scale=SCALE)

            nc.sync.dma_start(OUT[:], outstage[:])

    nc.compile()
    return nc


def _prep_inputs(x, conv1_w, conv1_b, conv2_w, conv2_b, fc1_w, fc1_b,
                 fc2_w, fc2_b, fc3_w, fc3_b):
    """Host-side preprocessing -> list of 8 per-core input dicts."""
    bf = ml_dtypes.bfloat16

    # conv1 weights: [128, 128]: 2 replicas of block-diag [64 = 4b x 16taps, 128]
    wc1 = np.zeros((128, 128), np.float32)
    w1 = conv1_w.reshape(32, 3, 3)  # [c, dy, dx]
    for sub in range(2):
        for dy in range(3):
            for dx in range(3):
                k = 4 * dy + dx
                for b4 in range(4):
                    wc1[64 * sub + 4 * k + b4, 32 * b4:32 * (b4 + 1)] = w1[:, dy, dx]
    wc1 = wc1.astype(bf)

    thr1 = (1.0 - conv1_b).astype(np.float32)          # [32]
    t1 = np.tile(thr1, 4).reshape(128, 1).astype(np.float32)
    b1s = (-(t1.astype(np.float64)) * SCALE).astype(np.float32)

    # conv2 weights, row-pair scheme: lhsT [96 = (3dx x 32ci), (4rho x 2row x 64co)]
    # output row (2q+row) uses input row (2q+rho) with tap dy = rho - row
    wc2r = np.zeros((96, 4, 2, 64), np.float32)
    for rho in range(4):
        for row in range(2):
            dy = rho - row
            if 0 <= dy <= 2:
                for dx in range(3):
                    wc2r[32 * dx:32 * (dx + 1), rho, row, :] = conv2_w[:, :, dy, dx].T
    wc2r = wc2r.reshape(96, 512).astype(bf)
    # pool2-h selection matrix: pooled[c] = s2[c] + s2[c+64]
    spool = np.concatenate([np.eye(64), np.eye(64)], axis=0).astype(bf)
    thr2 = (4.0 * (1.0 - conv2_b)).astype(np.float32)  # [64]
    t2 = np.tile(thr2, 2).reshape(128, 1).astype(np.float32)
    b2s = (-(t2.astype(np.float64)) * SCALE).astype(np.float32)

    # fc1 weights (pool-avg 1/4 folded in): SBUF layout [128 part, (h, r, u)]
    # c2buf partition p = 64*par + ch with par = h3 % 2; r = 8*(h3//2) + w3;
    # feat = ch*128 + h3*8 + w3; unit = 128h + u
    fw4 = (fc1_w.reshape(256, 64, 16, 8) * 0.25).transpose(1, 2, 3, 0)  # [ch, h3, w3, u]
    arr = fw4.reshape(64, 8, 2, 8, 256)          # [ch, m, par, w3, u]
    arr = arr.transpose(2, 0, 1, 3, 4).reshape(128, 64, 256)  # [p, r, u]
    fc1wt = np.zeros((128, 2, 64, 128), np.float32)
    fc1wt[:, 0] = arr[:, :, 0:128]
    fc1wt[:, 1] = arr[:, :, 128:256]
    fc1wt = fc1wt.reshape(128, 2 * 64 * 128)
    fc1whi = fc1wt.astype(bf)
    fc1wlo = (fc1wt - fc1whi.astype(np.float32)).astype(bf)
    fc1b = np.ascontiguousarray(fc1_b.reshape(2, 128).T).astype(np.float32)

    # fc2: lhsT [u3, u4]; hi/lo split
    l2 = np.ascontiguousarray(fc2_w.T).astype(np.float32)   # [256 u3, 128 u4]
    l2a_full = l2.astype(bf)
    l2b_full = (l2 - l2a_full.astype(np.float32)).astype(bf)
    def chunked(a):  # [256, 128] -> [128, 256] with chunk-major cols
        return np.ascontiguousarray(a.reshape(2, 128, 128).transpose(1, 0, 2).reshape(128, 256))
    l2a = chunked(l2a_full)
    l2b = chunked(l2b_full)
    fc2bias = fc2_b.reshape(128, 1).astype(np.float32)

    l3 = np.zeros((128, 16), np.float32)
    l3[:, 0:10] = fc3_w.T                  # [u4, 10]
    l3a = l3.astype(bf)
    l3b = (l3 - l3a.astype(np.float32)).astype(bf)
    fc3bias = np.zeros((16, 1), np.float32)
    fc3bias[0:10, 0] = fc3_b

    common = dict(wc1=wc1, b1s=b1s, t1=t1, wc2r=wc2r, spool=spool, b2s=b2s, t2=t2,
                  fc1whi=fc1whi, fc1wlo=fc1wlo, fc1b=fc1b,
                  fc2a=l2a, fc2b=l2b, fc2bias=fc2bias,
                  fc3a=l3a, fc3b=l3b, fc3bias=fc3bias)

    # x: pad to [BL+1, 68, 64] bf16 per core (1-pixel halo at (1,1))
    xs = x.reshape(B, H, W)
    in_maps = []
    for c in range(N_CORES):
        xc = xs[c * BL:(c + 1) * BL]
        xp = np.zeros((BL + 1, HP, WP), np.float32)
        xp[:BL, 1:H + 1, 1:W + 1] = xc
        m = dict(common)
        m["xp"] = xp.reshape(BL + 1, HP * WP).astype(bf)
        in_maps.append(m)
    return in_maps


_NC_CACHE = {}


def _get_nc():
    if "nc" not in _NC_CACHE:
        _NC_CACHE["nc"] = build_program()
    return _NC_CACHE["nc"]


def kernel(**inputs):
    nc = _get_nc()
    in_maps = _prep_inputs(**inputs)
    res = run_bass_kernel_spmd(nc, in_maps, core_ids=list(range(N_CORES)))
    outs = []
    for c in range(N_CORES):
        o = res.results[c]["out"]            # [16, NS*BL]
        o = o.reshape(16, NS, BL)[0:10]      # [10, NS, BL]
        outs.append(o.transpose(1, 2, 0))    # [NS, BL, 10]
    return np.concatenate(outs, axis=1).astype(np.float32)  # [NS, B, 10]



# revision 30
# speedup vs baseline: 1.0068x; 1.0068x over previous
"""AudioSNN forward pass on 8 Trainium2 NeuronCores (pure data parallel).

Host side: weight re-layout + padding (numpy). Device side: Bass/Tile kernel
per core over a 128-batch shard.
"""
import os
import sys
import numpy as np

for _p in ("/opt/trn_rl_repo", "/root/.axon_site/_ro/trn_rl_repo"):
    if os.path.isdir(_p) and _p not in sys.path:
        sys.path.insert(0, _p)

import ml_dtypes
from contextlib import ExitStack

import concourse.bass as bass
import concourse.tile as tile
from concourse import mybir, bacc
from concourse.bass_utils import run_bass_kernel_spmd

BF16 = mybir.dt.bfloat16
F32 = mybir.dt.float32
GT = mybir.AluOpType.is_gt
ADD = mybir.AluOpType.add
SUB = mybir.AluOpType.subtract
MUL = mybir.AluOpType.mult
SIGMOID = mybir.ActivationFunctionType.Sigmoid

N_CORES = 8
B = 1024
BL = B // N_CORES        # 128 batch per core
H, W = 64, 32            # conv1 image
HP, WP = H + 3, W + 3    # padded (67, 35); conv1 uses a 4x4 tap window
H2, W2 = 32, 16          # conv2 image (after pool1)
H2P, W2P = H2 + 2, W2 + 2  # 34, 18
H3, W3 = 16, 8           # after pool2
NS = 25
BETA = 0.95
SCALE = float(2.0 ** 96)  # sigmoid saturation scale (exact power of two)

BC = 32                  # conv2 batch-chunk
NCHUNK = BL // BC        # 4
P1COLS = H2P * W2P * BC  # 34*18*32 = 19584
GROUPS = BL // 4         # 32 conv1 groups (4 batches each)

# spike engine split: tiles with index % MOD == 0 go to ACT, rest to DVE
ACT_MOD1 = 2
ACT_MOD2 = 2


def _emit_c2pool(nc, spool, c2v, chunk, gt, pw2, g):
    """pool2-h for one 2-pair group: selection matmuls summing the two
    row-partition blocks (in-place into bank 0 of the group tile), then the
    pooled [128 = (h-par, ch), (b, w')] slice is transposed into c2buf."""
    for qq in range(2):
        nc.tensor.matmul(gt[64 * qq:64 * (qq + 1), 0:256], spool[:, 0:64],
                         pw2[:, 256 * qq:256 * (qq + 1)], start=True, stop=True,
                         tile_position=(0, 64 * qq), skip_group_check=True)
    gq = gt[:].rearrange("p (s b w) -> p s w b", s=4, b=BC)
    src = gq[:, 0:1, :, :].squeeze()
    dst = c2v[:, 8 * g:8 * (g + 1), chunk * BC:(chunk + 1) * BC]
    nc.vector.tensor_copy(dst, src)


def build_program():
    nc = bacc.Bacc()

    # ---- inputs (host-preprocessed layouts) ----
    XP = nc.declare_dram_parameter("xp", [BL + 1, HP * WP], BF16, isOutput=False)
    WC1 = nc.declare_dram_parameter("wc1", [128, 128], BF16, isOutput=False)
    B1S = nc.declare_dram_parameter("b1s", [128, 1], F32, isOutput=False)   # -SCALE*thr1
    T1 = nc.declare_dram_parameter("t1", [128, 1], F32, isOutput=False)     # thr1
    WC2R = nc.declare_dram_parameter("wc2r", [96, 4 * 128], BF16, isOutput=False)  # [(dx,ci), (rho, 2row x 64co)]
    SPOOL = nc.declare_dram_parameter("spool", [128, 64], BF16, isOutput=False)   # pool-h selection
    B2S = nc.declare_dram_parameter("b2s", [128, 1], F32, isOutput=False)   # -SCALE*thr2
    T2 = nc.declare_dram_parameter("t2", [128, 1], F32, isOutput=False)     # thr2
    FC1WHI = nc.declare_dram_parameter("fc1whi", [128, 2 * 64 * 128], BF16, isOutput=False)
    FC1WLO = nc.declare_dram_parameter("fc1wlo", [128, 2 * 64 * 128], BF16, isOutput=False)
    FC1B = nc.declare_dram_parameter("fc1b", [128, 2], F32, isOutput=False)
    FC2A = nc.declare_dram_parameter("fc2a", [128, 256], BF16, isOutput=False)  # lhsT hi [u3_low, (chunk, u4)]
    FC2B = nc.declare_dram_parameter("fc2b", [128, 256], BF16, isOutput=False)  # lo
    FC2BIAS = nc.declare_dram_parameter("fc2bias", [128, 1], F32, isOutput=False)
    FC3A = nc.declare_dram_parameter("fc3a", [128, 16], BF16, isOutput=False)   # lhsT hi [u4, 10->16]
    FC3B = nc.declare_dram_parameter("fc3b", [128, 16], BF16, isOutput=False)
    FC3BIAS = nc.declare_dram_parameter("fc3bias", [16, 1], F32, isOutput=False)

    OUT = nc.declare_dram_parameter("out", [16, NS * BL], F32, isOutput=True)

    dbg = os.environ.get("KERNEL_DEBUG", "")
    dbg_outs = {}
    if dbg:
        dbg_outs["p1"] = nc.declare_dram_parameter("dbg_p1", [128, P1COLS], BF16, isOutput=True)
        dbg_outs["c2"] = nc.declare_dram_parameter("dbg_c2", [128, 64 * BL], BF16, isOutput=True)
        dbg_outs["cur3"] = nc.declare_dram_parameter("dbg_cur3", [256, BL], F32, isOutput=True)

    with tile.TileContext(nc) as tc, ExitStack() as top:
        consts = top.enter_context(tc.tile_pool(name="consts", bufs=1))

        wc1 = consts.tile([128, 128], BF16, tag="wc1")
        nc.sync.dma_start(wc1[:], WC1[:])
        b1s = consts.tile([128, 1], F32, tag="b1s")
        nc.sync.dma_start(b1s[:], B1S[:])
        t1 = consts.tile([128, 1], F32, tag="t1")
        nc.sync.dma_start(t1[:], T1[:])
        wc2r = consts.tile([96, 4 * 128], BF16, tag="wc2r")
        nc.sync.dma_start(wc2r[:], WC2R[:])
        spool = consts.tile([128, 64], BF16, tag="spool")
        nc.sync.dma_start(spool[:], SPOOL[:])
        b2s = consts.tile([128, 1], F32, tag="b2s")
        nc.sync.dma_start(b2s[:], B2S[:])
        t2 = consts.tile([128, 1], F32, tag="t2")
        nc.sync.dma_start(t2[:], T2[:])

        # c2 accumulation buffer: [128 = 2x64ch, 64 rounds x 128 batch] bf16
        # (pool2 counts are small integers -> exact in bf16)
        c2buf = top.enter_context(tc.tile_pool(name="c2bufp", bufs=1)).tile(
            [128, 64 * BL], BF16, tag="c2buf")

        # fc1 weights (bf16 hi/lo split); DMAs are issued mid-conv (gpsimd
        # queue) so the 8.4 MB transfer overlaps conv compute without
        # delaying the conv1 input loads.
        fc1w_hi = consts.tile([128, 2 * 64 * 128], BF16, tag="fc1w_hi")
        fc1w_lo = consts.tile([128, 2 * 64 * 128], BF16, tag="fc1w_lo")

        # ------------- conv1 + spike1 + pool1 + conv2 + spike2 + pool2 -------------
        with ExitStack() as convs:
            p1pool = convs.enter_context(tc.tile_pool(name="p1pool", bufs=1))
            xrep_pool = convs.enter_context(tc.tile_pool(name="xrep", bufs=2))
            s1pool = convs.enter_context(tc.tile_pool(name="s1pool", bufs=2))
            pwpool = convs.enter_context(tc.tile_pool(name="pwpool", bufs=2))
            stgpool = convs.enter_context(tc.tile_pool(name="stgpool", bufs=1))
            c1ps = convs.enter_context(tc.tile_pool(name="c1ps", bufs=2, space="PSUM"))
            c2ps = convs.enter_context(tc.tile_pool(name="c2ps", bufs=2, space="PSUM"))
            s2pool = convs.enter_context(tc.tile_pool(name="s2pool", bufs=4))
            pw2pool = convs.enter_context(tc.tile_pool(name="pw2pool", bufs=4))

            # persistent double-buffered P1 + padded pool1 staging; pads are
            # zeroed once and never overwritten afterwards.
            p1bufs = [p1pool.tile([128, P1COLS], BF16, tag=f"p1_{i}", name=f"p1_{i}")
                      for i in range(2)]
            for i in range(2):
                pv = p1bufs[i][:].rearrange("p (b h w) -> p b h w", b=BC, h=H2P, w=W2P)
                nc.vector.memset(pv[0:96, :, 0, :], 0.0)
                nc.vector.memset(pv[0:96, :, H2P - 1, :], 0.0)
            # chunk-wide pool1 staging: [128 = (4b x 32ch), (8 blk x 32h x 18w)]
            stg3 = stgpool.tile([128, 8 * H2 * W2P], BF16, tag="stg3")
            nc.gpsimd.memset(stg3[:], 0.0)

            # explicit xr buffers (rows with dy=3/dx=3 are never re-loaded and
            # must stay finite -> one-time memset)
            XW = H * WP  # 2240
            xrbufs = [xrep_pool.tile([128, XW], BF16, tag=f"xrb_{i}", name=f"xrb_{i}")
                      for i in range(2)]
            for i in range(2):
                nc.gpsimd.memset(xrbufs[i][:], 0.0)

            for chunk in range(NCHUNK):
                # P1: [128 = 4 replicas x 32ch, (b, h2p, w2p)] bf16, pool-sums 0..4
                p1 = p1bufs[chunk % 2]
                p1v = p1[:].rearrange("p (b h w) -> p b h w", b=BC, h=H2P, w=W2P)
                stg = stg3
                stgv = stg[:].rearrange("p (blk q hp v) -> p blk q hp v",
                                        blk=8, q=4, hp=8, v=W2P)

                for gg in range(BC // 8):
                    # xr rows: 64*sub + 4*k + b; each row holds a contiguous
                    # 64x64 window of padded x starting at (dy, dx)
                    xr = xrbufs[(chunk * 4 + gg) % 2]
                    xrv = xr[:].rearrange("p (h w) -> p h w", h=H, w=WP)
                    # rows with dy=3 or dx=3 carry zero weights in wc1 -> left
                    # stale; only 12 of 16 rows per (sub, dy) are loaded.
                    xsrc = XP[:]
                    for sub_ in range(2):
                        for dy in range(3):
                            src = bass.AP(xsrc.tensor,
                                          xsrc.offset + (chunk * BC + gg * 8 + 4 * sub_) * (HP * WP) + dy * WP,
                                          [[1, 3], [HP * WP, 4], [1, XW]])
                            r0 = 64 * sub_ + 16 * dy
                            nc.sync.dma_start(xr[r0:r0 + 12, :], src)

                    for sub in range(2):
                        sb = 64 * sub
                        # conv1 + fused spike1/pool1-w:
                        #   pw = (u_even > thr1) + spike(u_odd)
                        pwsub = pwpool.tile([128, 1024], BF16, tag="pw")  # (4q,16h,16w')
                        pwsubv = pwsub[:].rearrange("p (q h w) -> p q h w", q=4, h=16)
                        for half in range(2):
                            hs = c1ps.tile([128, 1024], F32, tag="c1ps")
                            for qh in range(2):
                                q4 = 2 * half + qh
                                nc.tensor.matmul(hs[:, 512 * qh:512 * (qh + 1)],
                                                 wc1[sb:sb + 64, :],
                                                 xrv[sb:sb + 64, 16 * q4:16 * q4 + 16, 0:W],
                                                 start=True, stop=True,
                                                 tile_position=(sb, 0))
                            hsv = hs[:].rearrange("p (q h w) -> p q h w", q=2, h=16)
                            s1o = s1pool.tile([128, 512], BF16, tag="s1o")
                            s1ov = s1o[:].rearrange("p (q h w) -> p q h w", q=2, h=16)
                            nc.scalar.activation(s1ov, hsv[:, :, :, 1::2], SIGMOID,
                                                 bias=b1s[:], scale=SCALE)
                            nc.vector.scalar_tensor_tensor(
                                pwsubv[:, 2 * half:2 * half + 2], hsv[:, :, :, 0::2],
                                t1[:], s1ov, op0=GT, op1=ADD)
                        # pool1 h-pairs into padded staging block (GpSimd)
                        blk = 2 * gg + sub
                        pwq = pwsub[:].rearrange("p (q hp t w) -> p q hp t w",
                                                 q=4, hp=8, t=2)
                        nc.gpsimd.tensor_tensor(stgv[:, blk, :, :, 1:W2 + 1],
                                                pwq[:, :, :, 0, :], pwq[:, :, :, 1, :],
                                                op=ADD)

                    # scatter the finished gg block-pair into all three P1
                    # dx-replicas directly (dst shifted by -rep; the spill
                    # lands only in pad positions conv2 never reads).
                    # Spread across queues for DMA-engine parallelism.
                    qs = [nc.sync, nc.gpsimd, nc.scalar]
                    for rep in range(3):
                        for b4 in range(4):
                            off = (32 * rep) * P1COLS + b4 * (H2P * W2P) + W2P \
                                + 2 * gg * 4 * (H2P * W2P) - rep
                            dst = bass.AP(p1.tensor, p1.offset + off,
                                          [[P1COLS, 32], [4 * H2P * W2P, 2],
                                           [1, H2 * W2P]])
                            qs[rep].dma_start(
                                dst, stg[32 * b4:32 * (b4 + 1),
                                         1152 * gg:1152 * (gg + 1)])
                if dbg and chunk == 0:
                    nc.sync.dma_start(dbg_outs["p1"][:], p1[:])

                # ---- conv2: row-pair scheme, M=128 = (2 rows x 64 ch_out) ----
                # pooled-row-pair q covers output rows (2q, 2q+1); 4 accumulating
                # K=96 matmuls over input rows 2q+rho. Groups of 2 pairs share a
                # 2-bank psum tile; spike2 + pool2-w fuse into one ACT + one STT;
                # pool2-h is a PE matmul against a 2x64 selection matrix, written
                # in-place into the group tile's first bank.
                if chunk == 1:
                    # WAW dep on a conv-phase tile keeps the scheduler from
                    # hoisting the 4 MB transfers ahead of the conv1 inputs
                    nc.vector.tensor_copy(fc1w_hi[0:1, 0:1], stg3[0:1, 0:1])
                    nc.scalar.dma_start(fc1w_hi[:], FC1WHI[:])
                if chunk == 2:
                    nc.vector.tensor_copy(fc1w_lo[0:1, 0:1], stg3[0:1, 0:1])
                    nc.scalar.dma_start(fc1w_lo[:], FC1WLO[:])
                c2v = c2buf[:].rearrange("p (r b) -> p r b", b=BL)
                pend = None
                for g in range(8):
                    gt = c2ps.tile([128, 1024], F32, tag="c2g", name=f"c2g_{chunk}_{g}")
                    for qq in range(2):
                        q = 2 * g + qq
                        for rho in range(4):
                            nc.tensor.matmul(gt[:, 512 * qq:512 * (qq + 1)],
                                             wc2r[0:96, 128 * rho:128 * (rho + 1)],
                                             p1v[0:96, :, 2 * q + rho, 0:W2],
                                             start=(rho == 0), stop=(rho == 3))
                    gtv = gt[:].rearrange("p (qq b w) -> p qq b w", qq=2, b=BC)
                    s2o = s2pool.tile([128, 512], BF16, tag="s2o", name=f"s2o_{chunk}_{g}")
                    s2ov = s2o[:].rearrange("p (qq b w) -> p qq b w", qq=2, b=BC)
                    nc.scalar.activation(s2ov, gtv[:, :, :, 1::2], SIGMOID,
                                         bias=b2s[:], scale=SCALE)
                    pw2 = pw2pool.tile([128, 512], BF16, tag="pw2", name=f"pw2_{chunk}_{g}")
                    pw2v = pw2[:].rearrange("p (qq b w) -> p qq b w", qq=2, b=BC)
                    nc.vector.scalar_tensor_tensor(pw2v, gtv[:, :, :, 0::2], t2[:],
                                                   s2ov, op0=GT, op1=ADD)
                    if pend is not None:
                        _emit_c2pool(nc, spool, c2v, chunk, *pend)
                    pend = (gt, pw2, g)
                _emit_c2pool(nc, spool, c2v, chunk, *pend)

        if dbg:
            nc.sync.dma_start(dbg_outs["c2"][:], c2buf[:])

        # ---------------- fc1 (bf16 hi/lo) + LIF ----------------
        with ExitStack() as fcs:
            fc1ps = fcs.enter_context(tc.tile_pool(name="fc1ps", bufs=2, space="PSUM"))
            lifps = fcs.enter_context(tc.tile_pool(name="lifps", bufs=2, space="PSUM"))
            lifc = fcs.enter_context(tc.tile_pool(name="lifc", bufs=1))

            fc1b = consts.tile([128, 2], F32, tag="fc1b")
            nc.sync.dma_start(fc1b[:], FC1B[:])

            cur3c = lifc.tile([128, 2 * BL], F32, tag="cur3c")
            c2r = c2buf[:].rearrange("p (r b) -> p r b", b=BL)
            whiv = fc1w_hi[:].rearrange("p (h r u) -> p h r u", h=2, r=64)
            wlov = fc1w_lo[:].rearrange("p (h r u) -> p h r u", h=2, r=64)
            for h in range(2):
                ps = fc1ps.tile([128, BL], F32, tag="fc1ps", name=f"fc1ps_{h}")
                for r in range(64):
                    nc.tensor.matmul(ps[:], whiv[:, h, r, :], c2r[:, r, :],
                                     start=(r == 0), stop=False)
                    nc.tensor.matmul(ps[:], wlov[:, h, r, :], c2r[:, r, :],
                                     start=False, stop=(r == 63))
                # cur3 = psum + fc1_b (the 1/4 pool scale is folded into weights)
                nc.vector.tensor_scalar(cur3c[:, h * BL:(h + 1) * BL], ps[:],
                                        fc1b[:, h:h + 1], None, op0=ADD)
            if dbg:
                nc.sync.dma_start(dbg_outs["cur3"][0:128, :], cur3c[:, 0:BL])
                nc.sync.dma_start(dbg_outs["cur3"][128:256, :], cur3c[:, BL:2 * BL])

            # LIF weights
            fc2a = consts.tile([128, 256], BF16, tag="fc2a")
            nc.sync.dma_start(fc2a[:], FC2A[:])
            fc2b_w = consts.tile([128, 256], BF16, tag="fc2b_w")
            nc.sync.dma_start(fc2b_w[:], FC2B[:])
            fc2bias = consts.tile([128, 1], F32, tag="fc2bias")
            nc.sync.dma_start(fc2bias[:], FC2BIAS[:])
            fc3a = consts.tile([128, 16], BF16, tag="fc3a")
            nc.sync.dma_start(fc3a[:], FC3A[:])
            fc3b_w = consts.tile([128, 16], BF16, tag="fc3b_w")
            nc.sync.dma_start(fc3b_w[:], FC3B[:])
            fc3bias = consts.tile([16, 1], F32, tag="fc3bias")
            nc.sync.dma_start(fc3bias[:], FC3BIAS[:])

            # LIF chains, software-pipelined per step across engines:
            #   layer3 mem update: DVE; spk3: ACT (sigmoid saturation trick)
            #   fc2: PE; cur4 psum->SBUF copy: ACT
            #   layer4 chain: GpSimd (fc2 bias folded into the mem update)
            #   fc3: PE; layer5 chain: DVE (reads p5 psum directly)
            mem3 = lifc.tile([128, 2 * BL], F32, tag="mem3")
            t3 = lifc.tile([128, 2 * BL], F32, tag="t3")
            spk3buf = lifc.tile([128, NS * 2 * BL], BF16, tag="spk3buf")
            mem4 = lifc.tile([128, BL], F32, tag="mem4")
            t4 = lifc.tile([128, BL], F32, tag="t4")
            cur4buf = lifc.tile([128, NS * BL], F32, tag="cur4buf")
            spk4buf = lifc.tile([128, NS * BL], BF16, tag="spk4buf")
            mem5 = lifc.tile([16, BL], F32, tag="mem5")
            t5 = lifc.tile([16, BL], F32, tag="t5")
            outstage = lifc.tile([16, NS * BL], F32, tag="outstage")
            zero3 = lifc.tile([128, 2 * BL], BF16, tag="zero3")
            zero4 = lifc.tile([128, BL], BF16, tag="zero4")
            zero5 = lifc.tile([16, BL], F32, tag="zero5")
            for t_ in (mem3, mem4, mem5, zero3, zero4, zero5):
                nc.vector.memset(t_[:], 0.0)
            # bias tile for sigmoid(SCALE*(x-1)) spike trick
            nsig3 = lifc.tile([128, 1], F32, tag="nsig3")
            nc.vector.memset(nsig3[:], -float(SCALE))

            cur5buf = lifc.tile([16, NS * BL], F32, tag="cur5buf")
            for st in range(NS):
                # ---- layer 3: STT (DVE), sub (GpSimd), spike (ACT) ----
                nc.vector.scalar_tensor_tensor(t3[:], mem3[:], BETA, cur3c[:],
                                               op0=MUL, op1=ADD)
                prev3 = zero3[:] if st == 0 else spk3buf[:, (st - 1) * 2 * BL:st * 2 * BL]
                nc.gpsimd.tensor_tensor(mem3[:], t3[:], prev3, op=SUB)
                s3 = spk3buf[:, st * 2 * BL:(st + 1) * 2 * BL]
                nc.scalar.activation(s3, mem3[:], SIGMOID, bias=nsig3[:], scale=SCALE)
                # ---- fc2 (PE) ----
                p4 = lifps.tile([128, BL], F32, tag="p4", name=f"p4_{st}")
                s3a = spk3buf[:, st * 2 * BL:st * 2 * BL + BL]
                s3b = spk3buf[:, st * 2 * BL + BL:(st + 1) * 2 * BL]
                nc.tensor.matmul(p4[:], fc2a[:, 0:128], s3a, start=True, stop=False)
                nc.tensor.matmul(p4[:], fc2a[:, 128:256], s3b, start=False, stop=False)
                nc.tensor.matmul(p4[:], fc2b_w[:, 0:128], s3a, start=False, stop=False)
                nc.tensor.matmul(p4[:], fc2b_w[:, 128:256], s3b, start=False, stop=True)
                # ---- layer 4: bias (DVE), STT (DVE), sub (GpSimd), spike (ACT) ----
                cur4 = cur4buf[:, st * BL:(st + 1) * BL]
                nc.vector.tensor_scalar(cur4, p4[:], fc2bias[:, 0:1], None, op0=ADD)
                nc.vector.scalar_tensor_tensor(t4[:], mem4[:], BETA, cur4,
                                               op0=MUL, op1=ADD)
                prev4 = zero4[:] if st == 0 else spk4buf[:, (st - 1) * BL:st * BL]
                nc.gpsimd.tensor_tensor(mem4[:], t4[:], prev4, op=SUB)
                s4 = spk4buf[:, st * BL:(st + 1) * BL]
                nc.scalar.activation(s4, mem4[:], SIGMOID, bias=nsig3[:], scale=SCALE)
                # ---- fc3 (PE) + layer 5 (DVE/GpSimd/ACT) ----
                p5 = lifps.tile([16, BL], F32, tag="p5", name=f"p5_{st}")
                nc.tensor.matmul(p5[:], fc3a[:], s4, start=True, stop=False)
                nc.tensor.matmul(p5[:], fc3b_w[:], s4, start=False, stop=True)
                cur5 = cur5buf[:, st * BL:(st + 1) * BL]
                nc.vector.tensor_scalar(cur5, p5[:], fc3bias[:, 0:1], None, op0=ADD)
                nc.vector.scalar_tensor_tensor(t5[:], mem5[:], BETA, cur5,
                                               op0=MUL, op1=ADD)
                prev5 = zero5[:] if st == 0 else outstage[:, (st - 1) * BL:st * BL]
                nc.gpsimd.tensor_tensor(mem5[:], t5[:], prev5, op=SUB)
                nc.scalar.activation(outstage[:, st * BL:(st + 1) * BL],
                                     mem5[:], SIGMOID, bias=nsig3[0:16, :], scale=SCALE)

            nc.sync.dma_start(OUT[:], outstage[:])

    nc.compile()
    return nc


def _prep_inputs(x, conv1_w, conv1_b, conv2_w, conv2_b, fc1_w, fc1_b,
                 fc2_w, fc2_b, fc3_w, fc3_b):
    """Host-side preprocessing -> list of 8 per-core input dicts."""
    bf = ml_dtypes.bfloat16

    # conv1 weights: [128, 128]: 2 replicas of block-diag [64 = 4b x 16taps, 128]
    wc1 = np.zeros((128, 128), np.float32)
    w1 = conv1_w.reshape(32, 3, 3)  # [c, dy, dx]
    for sub in range(2):
        for dy in range(3):
            for dx in range(3):
                k = 4 * dy + dx
                for b4 in range(4):
                    wc1[64 * sub + 4 * k + b4, 32 * b4:32 * (b4 + 1)] = w1[:, dy, dx]
    wc1 = wc1.astype(bf)

    thr1 = (1.0 - conv1_b).astype(np.float32)          # [32]
    t1 = np.tile(thr1, 4).reshape(128, 1).astype(np.float32)
    b1s = (-(t1.astype(np.float64)) * SCALE).astype(np.float32)

    # conv2 weights, row-pair scheme: lhsT [96 = (3dx x 32ci), (4rho x 2row x 64co)]
    # output row (2q+row) uses input row (2q+rho) with tap dy = rho - row
    wc2r = np.zeros((96, 4, 2, 64), np.float32)
    for rho in range(4):
        for row in range(2):
            dy = rho - row
            if 0 <= dy <= 2:
                for dx in range(3):
                    wc2r[32 * dx:32 * (dx + 1), rho, row, :] = conv2_w[:, :, dy, dx].T
    wc2r = wc2r.reshape(96, 512).astype(bf)
    # pool2-h selection matrix: pooled[c] = s2[c] + s2[c+64]
    spool = np.concatenate([np.eye(64), np.eye(64)], axis=0).astype(bf)
    thr2 = (4.0 * (1.0 - conv2_b)).astype(np.float32)  # [64]
    t2 = np.tile(thr2, 2).reshape(128, 1).astype(np.float32)
    b2s = (-(t2.astype(np.float64)) * SCALE).astype(np.float32)

    # fc1 weights (pool-avg 1/4 folded in): SBUF layout [128 part, (h, r, u)]
    # c2buf partition p = 64*par + ch with par = h3 % 2; r = 8*(h3//2) + w3;
    # feat = ch*128 + h3*8 + w3; unit = 128h + u
    fw4 = (fc1_w.reshape(256, 64, 16, 8) * 0.25).transpose(1, 2, 3, 0)  # [ch, h3, w3, u]
    arr = fw4.reshape(64, 8, 2, 8, 256)          # [ch, m, par, w3, u]
    arr = arr.transpose(2, 0, 1, 3, 4).reshape(128, 64, 256)  # [p, r, u]
    fc1wt = np.zeros((128, 2, 64, 128), np.float32)
    fc1wt[:, 0] = arr[:, :, 0:128]
    fc1wt[:, 1] = arr[:, :, 128:256]
    fc1wt = fc1wt.reshape(128, 2 * 64 * 128)
    fc1whi = fc1wt.astype(bf)
    fc1wlo = (fc1wt - fc1whi.astype(np.float32)).astype(bf)
    fc1b = np.ascontiguousarray(fc1_b.reshape(2, 128).T).astype(np.float32)

    # fc2: lhsT [u3, u4]; hi/lo split
    l2 = np.ascontiguousarray(fc2_w.T).astype(np.float32)   # [256 u3, 128 u4]
    l2a_full = l2.astype(bf)
    l2b_full = (l2 - l2a_full.astype(np.float32)).astype(bf)
    def chunked(a):  # [256, 128] -> [128, 256] with chunk-major cols
        return np.ascontiguousarray(a.reshape(2, 128, 128).transpose(1, 0, 2).reshape(128, 256))
    l2a = chunked(l2a_full)
    l2b = chunked(l2b_full)
    fc2bias = fc2_b.reshape(128, 1).astype(np.float32)

    l3 = np.zeros((128, 16), np.float32)
    l3[:, 0:10] = fc3_w.T                  # [u4, 10]
    l3a = l3.astype(bf)
    l3b = (l3 - l3a.astype(np.float32)).astype(bf)
    fc3bias = np.zeros((16, 1), np.float32)
    fc3bias[0:10, 0] = fc3_b

    common = dict(wc1=wc1, b1s=b1s, t1=t1, wc2r=wc2r, spool=spool, b2s=b2s, t2=t2,
                  fc1whi=fc1whi, fc1wlo=fc1wlo, fc1b=fc1b,
                  fc2a=l2a, fc2b=l2b, fc2bias=fc2bias,
                  fc3a=l3a, fc3b=l3b, fc3bias=fc3bias)

    # x: pad to [BL+1, 68, 64] bf16 per core (1-pixel halo at (1,1))
    xs = x.reshape(B, H, W)
    in_maps = []
    for c in range(N_CORES):
        xc = xs[c * BL:(c + 1) * BL]
        xp = np.zeros((BL + 1, HP, WP), np.float32)
        xp[:BL, 1:H + 1, 1:W + 1] = xc
        m = dict(common)
        m["xp"] = xp.reshape(BL + 1, HP * WP).astype(bf)
        in_maps.append(m)
    return in_maps


_NC_CACHE = {}


def _get_nc():
    if "nc" not in _NC_CACHE:
        _NC_CACHE["nc"] = build_program()
    return _NC_CACHE["nc"]


def kernel(**inputs):
    nc = _get_nc()
    in_maps = _prep_inputs(**inputs)
    res = run_bass_kernel_spmd(nc, in_maps, core_ids=list(range(N_CORES)))
    outs = []
    for c in range(N_CORES):
        o = res.results[c]["out"]            # [16, NS*BL]
        o = o.reshape(16, NS, BL)[0:10]      # [10, NS, BL]
        outs.append(o.transpose(1, 2, 0))    # [NS, BL, 10]
    return np.concatenate(outs, axis=1).astype(np.float32)  # [NS, B, 10]

